# revision 18
# baseline (speedup 1.0000x reference)
"""Trainium2 Bass kernel for nn_EndPredictor (LN-GRU over B=256,T=512,D=1024,U=1024).

Data-parallel over batch: 32 rows/core x 8 cores. v2 design:
- 4-way fold: per-step tensors are (128,256/512), partition=32*g+batch,
  g=unit-group. Host permutes U/W gate columns so z|r interleave per group.
- Mean-fold: U and W columns are mean-centered on host (mean of h@U over
  columns is linear: h@u_bar), so the matmul output is exactly LN-centered
  and no mean stats are ever computed on device.
- LN variance: ScalarE Square+accum straight from PSUM (gates/cand) or DVE
  tensor_tensor_reduce on the fp16 staging copy (phase 1); one broadcast-
  combine matmul sums the 4 partition groups and replicates to 128
  partitions; rsqrt via fp-bit seed + fused 2-iteration Newton (custom DVE).
- Gates: s = clip((v*rinv)*gz + s1, 0, 1) as scalar_tensor_tensor + a custom
  fused CLIPADD DVE op. s1 ring stores the gz-prescaled phase-1 affine.
- h state is fp16-only; h_new = z*h + (1-z)*th computed in w-form so only
  two DVE ops trail the tanh; z*h runs on GPSIMD during the candidate MM.
- Phase 1 (s1 = affine(LN(x@W))) fp16, interleaved into the scan with a
  3-tile lead, SBUF s1 ring (no DRAM round-trip).
"""
import sys
for _p in ("/opt/trn_rl_repo", "/root/.axon_site/_ro/trn_rl_repo"):
    if _p not in sys.path:
        sys.path.insert(0, _p)

import numpy as np
import concourse.bass as bass
import concourse.bacc as bacc
import concourse.tile as tile
from concourse import mybir
from concourse.bass_utils import run_bass_kernel_spmd
from contextlib import ExitStack

F32 = mybir.dt.float32
I32 = mybir.dt.int32
FP16 = mybir.dt.float16
AF = mybir.ActivationFunctionType
OP = mybir.AluOpType

B, D, UNITS = 256, 1024, 1024
U3 = 3 * UNITS
Z2 = 2 * UNITS
NCORES = 8
BC = B // NCORES
EPS = 1e-5
MAGIC = 0x5F3759DF
MAGIC2 = MAGIC - (1 << 22)


# ---- runtime-registered custom DVE ops (self-pinned shas) ----
from concourse import dve_ops as _dvo
from concourse.dve_spec import (Spec as _Spec, Src0 as _S0, Src1 as _S1,
                                C0 as _C0, C1 as _C1, Zero as _Z0,
                                One as _One, sq as _sq, maxx as _maxx,
                                minn as _minn, lower as _lower,
                                _has_src1 as _hs1)
from concourse.dve_uop import DveOpSpec as _DveOpSpec


def _register_dve_op(name, spec, subdim=False):
    for _op in _dvo.OPS:
        if _op.name == name:
            return _op
    idx = _dvo._CUSTOM_DVE_ROW_BASE + len(_dvo.OPS)
    shas = {}
    for ver in ("v3", "v4"):
        s = _DveOpSpec(name=name, opcode=idx, uops=_lower(spec, ver=ver),
                       rd1_en=_hs1(spec))
        shas[ver] = s.sha(ver)
    op = _dvo.DveOp(name, spec, subdim, shas)
    _dvo.OPS.append(op)
    _dvo._SUB_OPCODE_FOR_NAME[name] = idx
    _dvo.CUSTOM_DVE_SPECS[name] = spec
    return op


# two Newton rsqrt iters on half-variance V' (Src0) from seed y0 (Src1), s0=1.5
_n2_y1 = _S1 * (_C0 - _S0 * _sq(_S1))


def _n2_ref(in0, in1, s0, s1, imm2):
    y1 = in1 * (s0 - in0 * in1 ** 2)
    return y1 * (s0 - in0 * y1 ** 2)


NEWTON2_RSQ = _register_dve_op("NEWTON2_RSQ_ANT", _Spec(
    body=_n2_y1 * (_C0 - _S0 * _sq(_n2_y1)), reference=_n2_ref))

# s = clip(Src0 + Src1, 0, 1)
CLIPADD = _register_dve_op("CLIPADD_ANT", _Spec(
    body=_minn(_maxx(_S0 + _S1, _Z0), _One),
    reference=lambda in0, in1, s0, s1, imm2: np.minimum(
        np.maximum(in0 + in1, 0.0), 1.0)))


T_STEPS = 512
TRACE = False
DBG_SCAN_STEPS = None
DBG_SKIP_P1 = False
DBG_STEP_LEVEL = 0   # 0=full; 1=gates MM+stats only; 2=+rsqrt; 3=+r/z/trans


def build_program(T, b1val, apply_mask=False, has_b=False):
    assert not has_b, "phase-1 bias add dropped (b==0 for this problem)"
    nc = bacc.Bacc("TRN2", target_bir_lowering=False, debug=False,
                   num_devices=NCORES)
    R = BC * T

    xt = nc.dram_tensor("xt", [D, R], FP16, kind="ExternalInput")
    w = nc.dram_tensor("w", [D, U3], FP16, kind="ExternalInput")   # permuted, mean-folded
    u = nc.dram_tensor("u", [D, U3], FP16, kind="ExternalInput")   # permuted, mean-folded
    afold = nc.dram_tensor("afold", [U3], FP16, kind="ExternalInput")
    cfold = nc.dram_tensor("cfold", [U3], FP16, kind="ExternalInput")
    gz = nc.dram_tensor("gz", [128, 512], F32, kind="ExternalInput")   # 4-way fold
    gc = nc.dram_tensor("gc", [128, 256], F32, kind="ExternalInput")   # 4-way fold
    a32 = nc.dram_tensor("a32", [128, 32], F32, kind="ExternalInput")
    a128 = nc.dram_tensor("a128", [128, 128], F32, kind="ExternalInput")
    idf = nc.dram_tensor("idf", [128, 128], FP16, kind="ExternalInput")
    w1f = nc.dram_tensor("w1f", [128, 256], F32, kind="ExternalInput")  # 4-way fold
    if apply_mask:
        mz = nc.dram_tensor("mz", [T, 128, 2], F32, kind="ExternalInput")
    out = nc.dram_tensor("out", [BC, 1], F32, kind="ExternalOutput")
    hdbg = nc.dram_tensor("hdbg", [128, 256], F32, kind="ExternalOutput")

    xt_r = xt.ap().rearrange("(k p) r -> p k r", k=8)
    w_r = w.ap().rearrange("(k p) c -> p k c", k=8)
    u_r = u.ap().rearrange("(k p) c -> p k c", k=8)

    def bcast_ap(h, n, cols):
        return bass.AP(tensor=h.ap().tensor, offset=0, ap=[[0, n], [1, cols]])

    with tile.TileContext(nc) as tc:
        ntiles = R // 128
        PRE = 3
        RING = 16
        with ExitStack() as p2:
            cons = p2.enter_context(tc.tile_pool(name="cons", bufs=1))
            xpool = p2.enter_context(tc.tile_pool(name="xpool", bufs=2))
            rawp = p2.enter_context(tc.tile_pool(name="rawp", bufs=1))
            p1sb = p2.enter_context(tc.tile_pool(name="p1sb", bufs=2))
            p1ps = p2.enter_context(tc.tile_pool(name="p1ps", bufs=2, space="PSUM"))
            sp = p2.enter_context(tc.tile_pool(name="scan_sb", bufs=4))
            hp = p2.enter_context(tc.tile_pool(name="hp", bufs=3))
            pz = p2.enter_context(tc.tile_pool(name="pz", bufs=2, space="PSUM"))
            pc = p2.enter_context(tc.tile_pool(name="pc", bufs=1, space="PSUM"))
            pt = p2.enter_context(tc.tile_pool(name="pt", bufs=1, space="PSUM"))
            pst = p2.enter_context(tc.tile_pool(name="pst", bufs=2, space="PSUM"))

            usb = cons.tile([128, 8, U3], FP16)
            nc.sync.dma_start(out=usb, in_=u_r)
            wsb = cons.tile([128, 8, U3], FP16)
            nc.sync.dma_start(out=wsb, in_=w_r)
            afb = cons.tile([128, U3], FP16)
            nc.sync.dma_start(out=afb, in_=bcast_ap(afold, 128, U3))
            cfb = cons.tile([128, U3], FP16)
            nc.sync.dma_start(out=cfb, in_=bcast_ap(cfold, 128, U3))
            s1z_ring = [cons.tile([128, 512], FP16, tag=f"s1zr{i}",
                                  name=f"s1zr{i}") for i in range(RING)]
            s1c_ring = [cons.tile([128, 256], FP16, tag=f"s1cr{i}",
                                  name=f"s1cr{i}") for i in range(RING)]
            gzsb = cons.tile([128, 512], F32)
            nc.sync.dma_start(out=gzsb, in_=gz.ap())
            gcsb = cons.tile([128, 256], F32)
            nc.sync.dma_start(out=gcsb, in_=gc.ap())
            a32sb = cons.tile([128, 32], F32)
            nc.sync.dma_start(out=a32sb, in_=a32.ap())
            a128sb = cons.tile([128, 128], F32)
            nc.sync.dma_start(out=a128sb, in_=a128.ap())
            idsb = cons.tile([128, 128], FP16)
            nc.sync.dma_start(out=idsb, in_=idf.ap())
            w1sb = cons.tile([128, 256], F32)
            nc.sync.dma_start(out=w1sb, in_=w1f.ap())
            b1b = cons.tile([32, 1], F32)
            nc.vector.memset(b1b, float(b1val))

            h16 = hp.tile([128, 256], FP16, tag="h16")
            hT = hp.tile([128, 256], FP16, tag="hT")
            nc.vector.memset(h16, 0.0)
            nc.vector.memset(hT, 0.0)

            def trans8(dst_ps, src16):
                """(128,256) 4-way-folded fp16 -> (128,256) unit-major
                transpose via identity matmuls."""
                for hf in range(2):
                    for jj in range(4):
                        k = 2 * jj + hf
                        nc.tensor.matmul(
                            dst_ps[:, 32 * k:32 * k + 32],
                            src16[:, 128 * hf:128 * hf + 128],
                            idsb[:, 32 * jj:32 * jj + 32],
                            start=True, stop=True)

            def rsqrt_from_ssq(ssq, inv2n, tag):
                """rinv (128,1) = rsqrt(sum-combine(ssq)*2*inv2n + eps) via
                combine matmul + bit seed + fused 2-iter Newton."""
                cp = pst.tile([128, 1], F32, tag="cstat")
                nc.tensor.matmul(cp, a128sb, ssq, start=True, stop=True)
                vp = sp.tile([128, 1], F32, tag=f"{tag}vp")
                nc.vector.tensor_scalar(out=vp, in0=cp, scalar1=inv2n,
                                        scalar2=EPS * 0.5, op0=OP.mult,
                                        op1=OP.add)
                y0i = sp.tile([128, 1], I32, tag=f"{tag}y0i")
                nc.vector.tensor_scalar(out=y0i, in0=vp.bitcast(I32),
                                        scalar1=-0.5, scalar2=float(MAGIC2),
                                        op0=OP.mult, op1=OP.add)
                rinv = sp.tile([128, 1], F32, tag=f"{tag}rinv")
                nc.vector._custom_dve(NEWTON2_RSQ, out=rinv, in0=vp,
                                      in1=y0i.bitcast(F32), s0=1.5)
                return rinv

            p1_state = {}

            p1_ps_half = {}

            def p1_mm_half(rt, n, half):
                """Half of GEMM chunk n (k 0-3 / 4-7); second half also does
                fp16 staging + sumsq + afold."""
                if n == 0 and half == 0:
                    xts = xpool.tile([128, 8, 128], FP16, tag="xts")
                    nc.sync.dma_start(
                        out=xts, in_=xt_r[:, :, rt * 128:(rt + 1) * 128])
                    stats = p1sb.tile([128, 6], F32, tag="p1stats")
                    p1_state[rt] = (xts, stats, [], None)
                xts, stats, raw2s, _ = p1_state[rt]
                if half == 0:
                    ps = p1ps.tile([128, 512], F32, tag="p1ps")
                    p1_ps_half[(rt, n)] = ps
                else:
                    ps = p1_ps_half.pop((rt, n))
                for k in range(4 * half, 4 * half + 4):
                    nc.tensor.matmul(ps, xts[:, k, :],
                                     wsb[:, k, n * 512:(n + 1) * 512],
                                     start=(k == 0), stop=(k == 7))
                if half == 0:
                    return
                raw = rawp.tile([128, 512], FP16, tag=f"p1raw{n % 2}")
                nc.scalar.activation(out=raw, in_=ps, func=AF.Copy)
                sqj = p1sb.tile([128, 512], FP16, tag="p1sqj")
                nc.scalar.activation(out=sqj, in_=ps, func=AF.Square,
                                     accum_out=stats[:, n:n + 1])
                raw2 = rawp.tile([128, 512], FP16, tag=f"p1raw2_{n}",
                                 name=f"p1raw2_{n}")
                nc.gpsimd.tensor_tensor(out=raw2, in0=raw,
                                        in1=afb[:, n * 512:(n + 1) * 512],
                                        op=OP.mult)
                raw2s.append(raw2)

            def p1_finish_stats(rt):
                """Stats -> rinv_y for row-tile rt (DVE smalls only)."""
                xts, stats, raw2s, _ = p1_state[rt]
                ssum = p1sb.tile([128, 1], F32, tag="p1ssum")
                nc.vector.tensor_reduce(out=ssum, in_=stats,
                                        axis=mybir.AxisListType.X, op=OP.add)
                vp = p1sb.tile([128, 1], F32, tag="p1vp")
                nc.vector.tensor_scalar(out=vp, in0=ssum,
                                        scalar1=1.0 / (2.0 * U3),
                                        scalar2=EPS * 0.5, op0=OP.mult,
                                        op1=OP.add)
                y0i = p1sb.tile([128, 1], I32, tag="p1y0i")
                nc.vector.tensor_scalar(out=y0i, in0=vp.bitcast(I32),
                                        scalar1=-0.5, scalar2=float(MAGIC2),
                                        op0=OP.mult, op1=OP.add)
                rv = p1sb.tile([128, 1], F32, tag="p1rv")
                nc.vector._custom_dve(NEWTON2_RSQ, out=rv, in0=vp,
                                      in1=y0i.bitcast(F32), s0=1.5)
                p1_state[rt] = (xts, stats, raw2s, rv)

            def p1_affine_chunk(rt, n):
                """Affine+scatter one chunk of row-tile rt (needs rinv_y)."""
                xts, stats, raw2s, rv = p1_state[rt]
                t0 = 4 * rt
                sbo = p1sb.tile([128, 512], FP16, tag=f"p1o{n % 3}")
                nc.vector.scalar_tensor_tensor(
                    out=sbo, in0=raw2s[n], scalar=rv,
                    in1=cfb[:, n * 512:(n + 1) * 512],
                    op0=OP.mult, op1=OP.add)
                for dt in range(4):
                    slot = (t0 + dt) % RING
                    if n < 4:
                        nc.sync.dma_start(
                            out=s1z_ring[slot][32 * n:32 * n + 32, :],
                            in_=sbo[32 * dt:32 * dt + 32, :])
                    else:
                        for hf in range(2):
                            jj = 2 * (n - 4) + hf
                            nc.sync.dma_start(
                                out=s1c_ring[slot][32 * jj:32 * jj + 32, :],
                                in_=sbo[32 * dt:32 * dt + 32,
                                        256 * hf:256 * hf + 256])
                if n == 5:
                    p1_state.pop(rt)

            def p1_work_for_step(t):
                """PE items (half-chunks) spread over 4 pop-points per step;
                DVE items (stats+affines) emitted at end of step."""
                if DBG_SKIP_P1:
                    return [], []
                pe_items, dve_items = [], []
                rt = t // 4 + PRE
                ph = t % 4
                if rt < ntiles:
                    if ph < 3:
                        pe_items = [("mm", rt, 2 * ph, 0), ("mm", rt, 2 * ph, 1),
                                    ("mm", rt, 2 * ph + 1, 0),
                                    ("mm", rt, 2 * ph + 1, 1)]
                    else:
                        dve_items = [("stats", rt, 0, 0), ("aff", rt, 0, 0),
                                     ("aff", rt, 1, 0)]
                rtp = rt - 1
                if PRE <= rtp < ntiles and rtp in p1_state:
                    if ph == 0:
                        dve_items += [("aff", rtp, 2, 0), ("aff", rtp, 3, 0)]
                    elif ph == 1:
                        dve_items += [("aff", rtp, 4, 0), ("aff", rtp, 5, 0)]
                return pe_items, dve_items

            def p1_emit(items):
                for kind, rt, n, half in items:
                    if kind == "mm":
                        p1_mm_half(rt, n, half)
                    elif kind == "stats":
                        p1_finish_stats(rt)
                    else:
                        p1_affine_chunk(rt, n)

            if not DBG_SKIP_P1:
                for rt in range(min(PRE, ntiles)):
                    for n in range(6):
                        p1_mm_half(rt, n, 0)
                        p1_mm_half(rt, n, 1)
                    p1_finish_stats(rt)
                    for n in range(6):
                        p1_affine_chunk(rt, n)

            nsteps = T if DBG_SCAN_STEPS is None else min(T, DBG_SCAN_STEPS)
            for t in range(nsteps):
                s1zt = s1z_ring[t % RING]
                s1ct = s1c_ring[t % RING]
                p1_pe, p1_dve = p1_work_for_step(t)

                # --- gates matmul (4-way col-tiled, k-accumulated) ---
                zrps = pz.tile([128, 512], F32, tag="zrps")
                for k in range(8):
                    for jj in range(4):
                        nc.tensor.matmul(zrps[32 * jj:32 * jj + 32, :],
                                         hT[:, 32 * k:32 * k + 32],
                                         usb[:, k, 512 * jj:512 * jj + 512],
                                         start=(k == 0), stop=(k == 7),
                                         tile_position=(0, 32 * jj),
                                         skip_group_check=True)
                # --- gates LN variance (ScalarE square+accum, mean-folded) ---
                sqjz = sp.tile([128, 512], F32, tag="sqjz")
                ssqz = sp.tile([128, 1], F32, tag="ssqz")
                nc.scalar.activation(out=sqjz, in_=zrps, func=AF.Square,
                                     accum_out=ssqz)
                # PE queue: half-chunk of p1 ahead of the combine-MM wait
                p1_emit(p1_pe[0:1])
                if DBG_STEP_LEVEL == 1:
                    hq = hp.tile([128, 256], FP16, tag="h16")
                    nc.vector.tensor_scalar_add(out=hq, in0=h16, scalar1=0.001)
                    hTq = hp.tile([128, 256], FP16, tag="hT")
                    nc.vector.tensor_copy(out=hTq, in_=hq)
                    h16, hT = hq, hTq
                    p1_emit(p1_pe[1:]); p1_emit(p1_dve)
                    continue
                rinv = rsqrt_from_ssq(ssqz, 1.0 / (2.0 * Z2), "z")
                # cover the r-elementwise window ahead of trans8's PE wait
                p1_emit(p1_pe[1:2])
                if DBG_STEP_LEVEL == 2:
                    hq = hp.tile([128, 256], FP16, tag="h16")
                    nc.vector.tensor_scalar(out=hq, in0=h16,
                                            scalar1=rinv, scalar2=0.001,
                                            op0=OP.mult, op1=OP.mult)
                    hTq = hp.tile([128, 256], FP16, tag="hT")
                    nc.vector.tensor_copy(out=hTq, in_=hq)
                    h16, hT = hq, hTq
                    p1_emit(p1_pe[2:]); p1_emit(p1_dve)
                    continue
                # --- r-half first: candidate chain waits only on r ---
                tr = sp.tile([128, 256], FP16, tag="tr")
                nc.vector.scalar_tensor_tensor(out=tr, in0=zrps[:, 256:512],
                                               scalar=rinv,
                                               in1=gzsb[:, 256:512],
                                               op0=OP.mult, op1=OP.mult)
                r = sp.tile([128, 256], FP16, tag="r")
                nc.vector._custom_dve(CLIPADD, out=r, in0=tr,
                                      in1=s1zt[:, 256:512])
                rh16 = sp.tile([128, 256], FP16, tag="rh16")
                nc.vector.tensor_tensor(out=rh16, in0=r, in1=h16, op=OP.mult)
                rhtps = pt.tile([128, 256], F32, tag="tps")
                trans8(rhtps, rh16)
                rhT = sp.tile([128, 256], FP16, tag="rhT")
                nc.vector.tensor_copy(out=rhT, in_=rhtps)
                # --- z-half (overlaps the candidate matmul) ---
                tz = sp.tile([128, 256], FP16, tag="tz")
                nc.vector.scalar_tensor_tensor(out=tz, in0=zrps[:, 0:256],
                                               scalar=rinv,
                                               in1=gzsb[:, 0:256],
                                               op0=OP.mult, op1=OP.mult)
                z = sp.tile([128, 256], FP16, tag="z")
                nc.vector._custom_dve(CLIPADD, out=z, in0=tz,
                                      in1=s1zt[:, 0:256])
                if apply_mask:
                    mzt = sp.tile([128, 2], F32, tag="mzt")
                    nc.sync.dma_start(out=mzt, in_=mz.ap()[t])
                    nc.vector.tensor_scalar(out=z, in0=z,
                                            scalar1=mzt[:, 0:1],
                                            scalar2=mzt[:, 1:2],
                                            op0=OP.mult, op1=OP.add)
                w_ = sp.tile([128, 256], FP16, tag="w_")
                nc.vector.tensor_scalar(out=w_, in0=z, scalar1=-1.0,
                                        scalar2=1.0, op0=OP.mult, op1=OP.add)
                zh = sp.tile([128, 256], FP16, tag="zh")
                nc.gpsimd.tensor_tensor(out=zh, in0=z, in1=h16, op=OP.mult)

                if DBG_STEP_LEVEL == 3:
                    hq = hp.tile([128, 256], FP16, tag="h16")
                    nc.vector.tensor_copy(out=hq, in_=rh16)
                    hTq = hp.tile([128, 256], FP16, tag="hT")
                    nc.vector.tensor_copy(out=hTq, in_=rhT)
                    h16, hT = hq, hTq
                    p1_emit(p1_pe[2:]); p1_emit(p1_dve)
                    continue
                # --- candidate matmul ---
                cps = pc.tile([128, 256], F32, tag="cps")
                for k in range(8):
                    for jj in range(4):
                        nc.tensor.matmul(
                            cps[32 * jj:32 * jj + 32, :],
                            rhT[:, 32 * k:32 * k + 32],
                            usb[:, k, Z2 + 256 * jj:Z2 + 256 * jj + 256],
                            start=(k == 0), stop=(k == 7),
                            tile_position=(0, 32 * jj),
                            skip_group_check=True)
                sqjc = sp.tile([128, 256], F32, tag="sqjc")
                ssqc = sp.tile([128, 1], F32, tag="ssqc")
                nc.scalar.activation(out=sqjc, in_=cps, func=AF.Square,
                                     accum_out=ssqc)
                # PE queue: p1 work ahead of the cand combine-MM wait
                p1_emit(p1_pe[2:3])
                rinvc = rsqrt_from_ssq(ssqc, 1.0 / (2.0 * UNITS), "c")
                # cover the tanh tail ahead of trans8's PE wait
                p1_emit(p1_pe[3:])
                tc_ = sp.tile([128, 256], FP16, tag="tc_")
                nc.vector.scalar_tensor_tensor(out=tc_, in0=cps,
                                               scalar=rinvc, in1=gcsb,
                                               op0=OP.mult, op1=OP.mult)
                cs = sp.tile([128, 256], FP16, tag="cs")
                nc.vector.tensor_tensor(out=cs, in0=tc_, in1=s1ct, op=OP.add)
                th = sp.tile([128, 256], FP16, tag="th")
                nc.scalar.activation(out=th, in_=cs, func=AF.Tanh)
                # --- h_new = z*h + (1-z)*th ---
                wth = sp.tile([128, 256], FP16, tag="wth")
                nc.vector.tensor_tensor(out=wth, in0=w_, in1=th, op=OP.mult)
                hn16 = hp.tile([128, 256], FP16, tag="h16")
                nc.vector.tensor_tensor(out=hn16, in0=zh, in1=wth, op=OP.add)
                htps = pt.tile([128, 256], F32, tag="tps")
                trans8(htps, hn16)
                hTn = hp.tile([128, 256], FP16, tag="hT")
                nc.vector.tensor_copy(out=hTn, in_=htps)
                h16, hT = hn16, hTn
                p1_emit(p1_dve)

            hf32 = sp.tile([128, 256], F32, tag="hf32")
            nc.vector.tensor_copy(out=hf32, in_=h16)
            pscr = sp.tile([128, 256], F32, tag="pscr")
            nc.vector.tensor_tensor(out=pscr, in0=hf32, in1=w1sb, op=OP.mult)
            pacc = sp.tile([128, 1], F32, tag="pacc")
            nc.vector.tensor_reduce(out=pacc, in_=pscr,
                                    axis=mybir.AxisListType.X, op=OP.add)
            cpo = pst.tile([32, 1], F32, tag="cstat")
            nc.tensor.matmul(cpo, a32sb, pacc, start=True, stop=True)
            ptot = sp.tile([32, 1], F32, tag="ptot")
            nc.vector.tensor_copy(out=ptot, in_=cpo)
            osb = sp.tile([32, 1], F32, tag="osb")
            nc.scalar.activation(out=osb, in_=ptot, func=AF.Sigmoid,
                                 bias=b1b, scale=1.0)
            nc.sync.dma_start(out=out.ap(), in_=osb)
            nc.sync.dma_start(out=hdbg.ap(), in_=hf32)

    nc.compile()
    return nc


def _perm_cols():
    """Column permutation: perm[i] = original column for permuted col i."""
    perm = np.empty(U3, np.int64)
    for jj in range(4):
        perm[512 * jj:512 * jj + 256] = np.arange(256 * jj, 256 * jj + 256)
        perm[512 * jj + 256:512 * jj + 512] = np.arange(
            1024 + 256 * jj, 1024 + 256 * jj + 256)
    perm[Z2:] = np.arange(Z2, U3)
    return perm


def _host_prep(x, mask, W, U, b, gammas, betas, W1, b1, T, apply_mask, has_b):
    perm = _perm_cols()
    g0 = np.asarray(gammas[0], np.float64)[perm]
    g1 = np.asarray(gammas[1], np.float64)[perm]
    be0 = np.asarray(betas[0], np.float64)[perm]
    be1 = np.asarray(betas[1], np.float64)[perm]
    # s1_scaled = rinv_y * (ybar * afold) + cfold, with
    # afold = [0.2*g0 | g0], cfold = [0.2*(be0+be1)+0.5 | be0+be1]
    afold = np.where(np.arange(U3) < Z2, 0.2 * g0, g0).astype(np.float32)
    cfold = np.where(np.arange(U3) < Z2,
                     0.2 * (be0 + be1) + 2.5 * 0.2, be0 + be1).astype(np.float32)
    gzv = (0.2 * g1[:Z2]).astype(np.float32)
    gcv = g1[Z2:].astype(np.float32)
    gzf = np.ascontiguousarray(
        np.broadcast_to(gzv.reshape(4, 1, 512), (4, 32, 512)).reshape(128, 512))
    gcf = np.ascontiguousarray(
        np.broadcast_to(gcv.reshape(4, 1, 256), (4, 32, 256)).reshape(128, 256))
    a32 = np.zeros((128, 32), np.float32)
    a32[np.arange(128), np.arange(128) % 32] = 1.0
    a128 = np.ascontiguousarray(np.tile(np.eye(32, dtype=np.float32), (4, 4)))
    idf = np.eye(128, dtype=np.float16)
    # Mean-fold: U gate/cand sections and W (full 3072) centered over columns.
    Wp = np.asarray(W, np.float64)[:, perm]
    Up = np.asarray(U, np.float64)[:, perm]
    Wp = Wp - Wp.mean(axis=1, keepdims=True)
    Up[:, :Z2] -= Up[:, :Z2].mean(axis=1, keepdims=True)
    Up[:, Z2:] -= Up[:, Z2:].mean(axis=1, keepdims=True)
    shared = dict(w=np.ascontiguousarray(Wp.astype(np.float16)),
                  u=np.ascontiguousarray(Up.astype(np.float16)),
                  afold=afold.astype(np.float16), cfold=cfold.astype(np.float16),
                  gz=gzf, gc=gcf,
                  a32=a32, a128=a128, idf=idf,
                  w1f=np.ascontiguousarray(np.broadcast_to(
                      np.asarray(W1, np.float32).reshape(4, 1, 256),
                      (4, 32, 256)).reshape(128, 256)))
    in_maps = []
    for c in range(NCORES):
        xc = x[c * BC:(c + 1) * BC, :T, :]
        xtc = np.ascontiguousarray(
            np.transpose(xc, (2, 1, 0)).reshape(D, T * BC).astype(np.float16))
        m = dict(shared)
        m["xt"] = xtc
        if apply_mask:
            mc = np.asarray(mask[c * BC:(c + 1) * BC, :T], np.float32)  # (32,T)
            mzt = np.empty((T, 128, 2), np.float32)
            for jj in range(4):
                mzt[:, 32 * jj:32 * jj + 32, 0] = mc.T
            mzt[:, :, 1] = 1.0 - mzt[:, :, 0]
            m["mz"] = mzt
        in_maps.append(m)
    return in_maps


def kernel(x, mask, W, U, b, gammas, betas, W1, b1):
    import time as _time
    x = np.asarray(x)
    T = x.shape[1]
    has_b = bool(np.any(np.asarray(b)))
    apply_mask = not bool(np.asarray(mask).all())
    b1val = float(np.asarray(b1).reshape(-1)[0])
    _t0 = _time.time()
    nc = build_program(T, b1val, apply_mask=apply_mask, has_b=has_b)
    _t1 = _time.time()
    in_maps = _host_prep(x, mask, W, U, b, gammas, betas, W1, b1, T,
                         apply_mask, has_b)
    _t2 = _time.time()
    res = run_bass_kernel_spmd(nc, in_maps, core_ids=list(range(NCORES)),
                               trace=TRACE)
    _t3 = _time.time()
    print(f"[kernel] build {_t1-_t0:.1f}s prep {_t2-_t1:.1f}s run {_t3-_t2:.1f}s")
    kernel.last_result = res
    kernel.last_nc = nc
    kernel.last_in_maps = in_maps
    outs = [res.results[c]["out"].reshape(BC, 1) for c in range(NCORES)]
    return np.concatenate(outs, axis=0).astype(np.float32)
